# revision 4
# baseline (speedup 1.0000x reference)
"""MeshPool kernel for 8x TRN2 NeuronCores.

out = segment_sum(vals[:,None] * x[cols], rows, M) / segment_sum(vals, rows, M)

Structure exploited (from the reference generator): every output row m has
exactly 4 COO entries (rows = arange(NNZ) % M), cols is a permutation. We
verify this at runtime via a generic grouping pass.

Strategy (no collectives): shard output rows across 8 cores (3125 each,
padded to 3200 = 25 tiles x 128). Each core gathers the x-rows it needs with
SWDGE dma_gather (int16 indices => x split into 4 chunks of 25000 rows),
then routes each gathered row to its output row with a one-hot weight matrix
W (built on DVE from per-entry (target,weight) descriptors) and a PSUM
accumulated matmul:  out_tile[128,256] = sum_c W_c.T @ G_c.  The division is
folded into host-precomputed weights w = vals/den (f64 host precision).
"""

import numpy as np

M_COARSE = 25000
N_FINE = 100000
D = 256
NNZ = 100000
NCORES = 8
NCHUNK = 4
CHUNK = 25000          # x rows per chunk (int16 gather index < 32768)
TILE = 128             # output rows per tile
TILES_PER_CORE = 25
GROUP_TILES = 5        # tiles per gather group
GROUPS = TILES_PER_CORE // GROUP_TILES
ROWS_PER_CORE = TILES_PER_CORE * TILE          # 3200 padded row slots
IDX_COLS = ROWS_PER_CORE // 16                 # 200 wrapped idx columns/chunk
GIDX = GROUP_TILES * TILE                      # 640 idxs per gather

MM_DTYPE = "float32"  # matmul dtype: float32r (1cyc/row) vs float32 (4cyc/row)

_COMPILED = None  # (nc, names) cache — NEFF is shape-only


# ----------------------------------------------------------------- planning
def _plan(rows, cols, vals):
    """Assign output rows to (core, tile, slot) and build per-core device
    inputs. Returns list of per-core dicts + m_of maps for unsharding."""
    rows = np.asarray(rows).astype(np.int64)
    cols = np.asarray(cols).astype(np.int64)
    vals64 = np.asarray(vals).astype(np.float64)

    # group entries by output row (generic, stable)
    order = np.argsort(rows, kind="stable")
    rs = rows[order]
    counts = np.bincount(rs, minlength=M_COARSE)
    assert counts.max() <= 4 and counts.min() >= 1, "kernel assumes <=4 nnz/row"
    den = np.zeros(M_COARSE)
    np.add.at(den, rows, vals64)
    w64 = vals64 / den[rows]                    # per-entry weight, f64
    starts = np.zeros(M_COARSE + 1, np.int64)
    np.cumsum(counts, out=starts[1:])

    ch = cols // CHUNK                          # chunk of each entry
    loc = (cols % CHUNK).astype(np.int64)       # local idx within chunk

    # per-row chunk profiles [M, 4]
    prof = np.zeros((M_COARSE, NCHUNK), np.int32)
    np.add.at(prof, (rows, ch), 1)

    rng = np.random.default_rng(0)

    # --- assign rows to cores, balancing per-chunk totals (skewed first,
    # minimize resulting max chunk load)
    skew = prof.max(axis=1)
    perm = np.argsort(-(skew * 100000 + rng.integers(0, 99999, M_COARSE)))
    core_rows = [[] for _ in range(NCORES)]
    core_load = np.zeros((NCORES, NCHUNK), np.int64)
    core_n = np.zeros(NCORES, np.int64)
    per_core = M_COARSE // NCORES
    for m in perm:
        cand = np.flatnonzero(core_n < per_core)
        k = cand[np.argmin((core_load[cand] + prof[m]).max(axis=1) * 10000
                           + core_load[cand].sum(axis=1))]
        core_rows[k].append(m)
        core_load[k] += prof[m]
        core_n[k] += 1
    assert core_load.max() <= TILES_PER_CORE * TILE, core_load.max()

    shards = []
    for k in range(NCORES):
        ms = np.array(core_rows[k])
        # --- assign rows to tiles (cap 128 rows, 128 entries/chunk)
        caps = np.full((TILES_PER_CORE, NCHUNK), TILE, np.int64)
        rcap = np.full(TILES_PER_CORE, TILE, np.int64)
        # most-skewed rows first
        sk = prof[ms].max(axis=1)
        for attempt in range(8):
            ordi = np.argsort(-(sk * 1000 + rng.integers(0, 999, len(ms))))
            caps[:] = TILE
            rcap[:] = TILE
            tile_of = np.full(len(ms), -1, np.int64)
            ok = True
            for i in ordi:
                p = prof[ms[i]]
                feas = (caps >= p).all(axis=1) & (rcap > 0)
                if not feas.any():
                    ok = False
                    break
                slack = (caps - p).min(axis=1) * 1000 + rcap
                slack[~feas] = -1
                t = int(np.argmax(slack))
                tile_of[i] = t
                caps[t] -= p
                rcap[t] -= 1
            if ok:
                break
        assert ok, "tile packing failed"

        idx16 = np.zeros((NCHUNK, ROWS_PER_CORE), np.int16)
        mt = np.zeros((NCHUNK, ROWS_PER_CORE), np.float32)
        wt = np.zeros((NCHUNK, ROWS_PER_CORE), np.float32)
        m_of = np.full(ROWS_PER_CORE, -1, np.int64)
        fill = np.zeros((TILES_PER_CORE, NCHUNK), np.int64)
        rfill = np.zeros(TILES_PER_CORE, np.int64)
        for i, m in enumerate(ms):
            t = tile_of[i]
            j = rfill[t]
            rfill[t] += 1
            m_of[t * TILE + j] = m
            for e in order[starts[m]:starts[m + 1]]:
                c = ch[e]
                p = fill[t, c]
                fill[t, c] += 1
                pos = t * TILE + p
                idx16[c, pos] = loc[e]
                mt[c, pos] = float(j)
                wt[c, pos] = np.float32(w64[e])

        # wrapped idx layout [128, 200] per chunk: idx i -> (i%16, i//16), x8 replicas
        wrapped = np.zeros((NCHUNK, 128, IDX_COLS), np.int16)
        for c in range(NCHUNK):
            resh = idx16[c].reshape(IDX_COLS, 16)     # [s, i%16]
            wrapped[c] = np.tile(resh.T, (8, 1))
        # mt/wt SBUF layout [128, 100]: col t*4+c, partition p
        mt_s = np.zeros((128, TILES_PER_CORE * NCHUNK), np.float32)
        wt_s = np.zeros((128, TILES_PER_CORE * NCHUNK), np.float32)
        for c in range(NCHUNK):
            v = mt[c].reshape(TILES_PER_CORE, TILE)   # [t, p]
            mt_s[:, c::NCHUNK] = v.T
            v = wt[c].reshape(TILES_PER_CORE, TILE)
            wt_s[:, c::NCHUNK] = v.T
        shards.append({"idxs": wrapped, "mt": mt_s, "wt": wt_s, "m_of": m_of})
    return shards


# ------------------------------------------------------------------- kernel
def _build():
    import concourse.bacc as bacc
    import concourse.mybir as mybir
    from concourse.tile import TileContext

    f32 = mybir.dt.float32
    mmdt = getattr(mybir.dt, MM_DTYPE)

    nc = bacc.Bacc("TRN2", target_bir_lowering=False, debug=False)
    x = nc.dram_tensor("x", [N_FINE, D], f32, kind="ExternalInput")
    idxs = nc.dram_tensor("idxs", [NCHUNK, 128, IDX_COLS], mybir.dt.int16,
                          kind="ExternalInput")
    mt = nc.dram_tensor("mt", [128, TILES_PER_CORE * NCHUNK], f32,
                        kind="ExternalInput")
    wt = nc.dram_tensor("wt", [128, TILES_PER_CORE * NCHUNK], f32,
                        kind="ExternalInput")
    y = nc.dram_tensor("y", [ROWS_PER_CORE, D], f32, kind="ExternalOutput")

    with TileContext(nc) as tc:
        with (
            tc.tile_pool(name="const", bufs=1) as cpool,
            tc.tile_pool(name="g", bufs=2) as gpool,
            tc.tile_pool(name="w", bufs=2) as wpool,
            tc.tile_pool(name="o", bufs=2) as opool,
            tc.tile_pool(name="ps", bufs=2, space="PSUM") as ppool,
        ):
            idx_sb = cpool.tile([128, NCHUNK * IDX_COLS], mybir.dt.int16)
            mt_sb = cpool.tile([128, TILES_PER_CORE * NCHUNK], f32)
            wt_sb = cpool.tile([128, TILES_PER_CORE * NCHUNK], f32)
            iota_i = cpool.tile([128, TILE], mybir.dt.int32)
            iota_f = cpool.tile([128, TILE], f32)
            for c in range(NCHUNK):
                nc.sync.dma_start(
                    out=idx_sb[:, c * IDX_COLS:(c + 1) * IDX_COLS],
                    in_=idxs[c, :, :])
            nc.sync.dma_start(out=mt_sb[:], in_=mt[:, :])
            nc.sync.dma_start(out=wt_sb[:], in_=wt[:, :])
            nc.gpsimd.iota(iota_i[:], pattern=[[1, TILE]], base=0,
                           channel_multiplier=0)
            nc.vector.tensor_copy(iota_f[:], iota_i[:])

            for g in range(GROUPS):
                G = []
                for c in range(NCHUNK):
                    gt = gpool.tile([128, GROUP_TILES * D], f32, tag=f"G{c}")
                    nc.gpsimd.dma_gather(
                        gt[:].rearrange("p (s d) -> p s d", d=D),
                        x[c * CHUNK:(c + 1) * CHUNK, :],
                        idx_sb[:, c * IDX_COLS + g * (GIDX // 16):
                               c * IDX_COLS + (g + 1) * (GIDX // 16)],
                        GIDX, GIDX, D)
                    G.append(gt)
                ostage = opool.tile([128, GROUP_TILES * D], f32, tag="out")
                for t5 in range(GROUP_TILES):
                    t = g * GROUP_TILES + t5
                    ps = ppool.tile([128, D], f32, tag="ps")
                    for c in range(NCHUNK):
                        W = wpool.tile([128, TILE], f32, tag=f"W{c}")
                        col = t * NCHUNK + c
                        nc.vector.tensor_scalar(
                            W[:], iota_f[:],
                            mt_sb[:, col:col + 1], wt_sb[:, col:col + 1],
                            mybir.AluOpType.is_equal, mybir.AluOpType.mult)
                        nc.tensor.matmul(
                            ps[:],
                            lhsT=W[:].bitcast(mmdt),
                            rhs=G[c][:, t5 * D:(t5 + 1) * D].bitcast(mmdt),
                            start=(c == 0), stop=(c == NCHUNK - 1))
                    nc.scalar.copy(ostage[:, t5 * D:(t5 + 1) * D], ps[:])
                nc.sync.dma_start(
                    out=y[g * GIDX:(g + 1) * GIDX, :].rearrange(
                        "(t p) d -> p t d", p=128),
                    in_=ostage[:].rearrange("p (t d) -> p t d", d=D))
    nc.compile()
    return nc


def _get_compiled():
    global _COMPILED
    if _COMPILED is None:
        _COMPILED = _build()
    return _COMPILED


# -------------------------------------------------------------------- entry
def kernel(x, vals, rows, cols):
    x = np.ascontiguousarray(np.asarray(x, dtype=np.float32))
    shards = _plan(rows, cols, vals)
    nc = _get_compiled()

    from concourse.bass_utils import run_bass_kernel_spmd
    in_maps = [
        {"x": x, "idxs": s["idxs"], "mt": s["mt"], "wt": s["wt"]}
        for s in shards
    ]
    res = run_bass_kernel_spmd(nc, in_maps, core_ids=list(range(NCORES)))

    out = np.zeros((M_COARSE, D), np.float32)
    for k, s in enumerate(shards):
        yk = res.results[k]["y"]
        valid = s["m_of"] >= 0
        out[s["m_of"][valid]] = yk[valid]
    return out


# revision 6
# speedup vs baseline: 1.5977x; 1.5977x over previous
"""MeshPool kernel for 8x TRN2 NeuronCores.

out = segment_sum(vals[:,None] * x[cols], rows, M) / segment_sum(vals, rows, M)

Structure exploited (from the reference generator): every output row m has
exactly 4 COO entries (rows = arange(NNZ) % M), cols is a permutation. We
verify this at runtime via a generic grouping pass.

Strategy (no collectives): shard output rows across 8 cores (3125 each,
padded to 3200 = 25 tiles x 128). Each core gathers the x-rows it needs with
SWDGE dma_gather (int16 indices => x split into 4 chunks of 25000 rows),
then routes each gathered row to its output row with a one-hot weight matrix
W (built on DVE from per-entry (target,weight) descriptors) and a PSUM
accumulated matmul:  out_tile[128,256] = sum_c W_c.T @ G_c.  The division is
folded into host-precomputed weights w = vals/den (f64 host precision).
"""

import numpy as np

M_COARSE = 25000
N_FINE = 100000
D = 256
NNZ = 100000
NCORES = 8
NCHUNK = 4
CHUNK = 25000          # x rows per chunk (int16 gather index < 32768)
TILE = 128             # output rows per tile
TILES_PER_CORE = 25
GROUP_TILES = 5        # tiles per gather group
GROUPS = TILES_PER_CORE // GROUP_TILES
ROWS_PER_CORE = TILES_PER_CORE * TILE          # 3200 padded row slots
IDX_COLS = ROWS_PER_CORE // 16                 # 200 wrapped idx columns/chunk
GIDX = GROUP_TILES * TILE                      # 640 idxs per gather

MM_DTYPE = "float32r"  # matmul dtype: float32r (1cyc/row) vs float32 (4cyc/row)

_COMPILED = None  # (nc, names) cache — NEFF is shape-only


# ----------------------------------------------------------------- planning
def _plan(rows, cols, vals):
    """Assign output rows to (core, tile, slot) and build per-core device
    inputs. Returns list of per-core dicts + m_of maps for unsharding."""
    rows = np.asarray(rows).astype(np.int64)
    cols = np.asarray(cols).astype(np.int64)
    vals64 = np.asarray(vals).astype(np.float64)

    # group entries by output row (generic, stable)
    order = np.argsort(rows, kind="stable")
    rs = rows[order]
    counts = np.bincount(rs, minlength=M_COARSE)
    assert counts.max() <= 4 and counts.min() >= 1, "kernel assumes <=4 nnz/row"
    den = np.zeros(M_COARSE)
    np.add.at(den, rows, vals64)
    w64 = vals64 / den[rows]                    # per-entry weight, f64
    starts = np.zeros(M_COARSE + 1, np.int64)
    np.cumsum(counts, out=starts[1:])

    ch = cols // CHUNK                          # chunk of each entry
    loc = (cols % CHUNK).astype(np.int64)       # local idx within chunk

    # per-row chunk profiles [M, 4]
    prof = np.zeros((M_COARSE, NCHUNK), np.int32)
    np.add.at(prof, (rows, ch), 1)

    rng = np.random.default_rng(0)

    # --- assign rows to cores, balancing per-chunk totals (skewed first,
    # minimize resulting max chunk load)
    skew = prof.max(axis=1)
    perm = np.argsort(-(skew * 100000 + rng.integers(0, 99999, M_COARSE)))
    core_rows = [[] for _ in range(NCORES)]
    core_load = np.zeros((NCORES, NCHUNK), np.int64)
    core_n = np.zeros(NCORES, np.int64)
    per_core = M_COARSE // NCORES
    for m in perm:
        cand = np.flatnonzero(core_n < per_core)
        k = cand[np.argmin((core_load[cand] + prof[m]).max(axis=1) * 10000
                           + core_load[cand].sum(axis=1))]
        core_rows[k].append(m)
        core_load[k] += prof[m]
        core_n[k] += 1
    assert core_load.max() <= TILES_PER_CORE * TILE, core_load.max()

    shards = []
    for k in range(NCORES):
        ms = np.array(core_rows[k])
        # --- assign rows to tiles (cap 128 rows, 128 entries/chunk)
        caps = np.full((TILES_PER_CORE, NCHUNK), TILE, np.int64)
        rcap = np.full(TILES_PER_CORE, TILE, np.int64)
        # most-skewed rows first
        sk = prof[ms].max(axis=1)
        for attempt in range(8):
            ordi = np.argsort(-(sk * 1000 + rng.integers(0, 999, len(ms))))
            caps[:] = TILE
            rcap[:] = TILE
            tile_of = np.full(len(ms), -1, np.int64)
            ok = True
            for i in ordi:
                p = prof[ms[i]]
                feas = (caps >= p).all(axis=1) & (rcap > 0)
                if not feas.any():
                    ok = False
                    break
                slack = (caps - p).min(axis=1) * 1000 + rcap
                slack[~feas] = -1
                t = int(np.argmax(slack))
                tile_of[i] = t
                caps[t] -= p
                rcap[t] -= 1
            if ok:
                break
        assert ok, "tile packing failed"

        idx16 = np.zeros((NCHUNK, ROWS_PER_CORE), np.int16)
        mt = np.zeros((NCHUNK, ROWS_PER_CORE), np.float32)
        wt = np.zeros((NCHUNK, ROWS_PER_CORE), np.float32)
        m_of = np.full(ROWS_PER_CORE, -1, np.int64)
        fill = np.zeros((TILES_PER_CORE, NCHUNK), np.int64)
        rfill = np.zeros(TILES_PER_CORE, np.int64)
        for i, m in enumerate(ms):
            t = tile_of[i]
            j = rfill[t]
            rfill[t] += 1
            m_of[t * TILE + j] = m
            for e in order[starts[m]:starts[m + 1]]:
                c = ch[e]
                p = fill[t, c]
                fill[t, c] += 1
                pos = t * TILE + p
                idx16[c, pos] = loc[e]
                mt[c, pos] = float(j)
                wt[c, pos] = np.float32(w64[e])

        # wrapped idx layout [128, 200] per chunk: idx i -> (i%16, i//16), x8 replicas
        wrapped = np.zeros((NCHUNK, 128, IDX_COLS), np.int16)
        for c in range(NCHUNK):
            resh = idx16[c].reshape(IDX_COLS, 16)     # [s, i%16]
            wrapped[c] = np.tile(resh.T, (8, 1))
        # dense routing matrices Wd[t*4+c, p, j] = weight
        Wd = np.zeros((TILES_PER_CORE * NCHUNK, 128, 128), np.float32)
        for c in range(NCHUNK):
            pos = np.arange(ROWS_PER_CORE)
            tc_i = (pos // TILE) * NCHUNK + c
            j_i = mt[c].astype(np.int64)
            Wd[tc_i, pos % TILE, j_i] = wt[c]
        shards.append({"idxs": wrapped, "wm": Wd, "m_of": m_of})
    return shards


# ------------------------------------------------------------------- kernel
def _build():
    import concourse.bacc as bacc
    import concourse.mybir as mybir
    from concourse.tile import TileContext

    f32 = mybir.dt.float32
    mmdt = getattr(mybir.dt, MM_DTYPE)

    nc = bacc.Bacc("TRN2", target_bir_lowering=False, debug=False,
                   num_swdge_queues=4)
    x = nc.dram_tensor("x", [N_FINE, D], f32, kind="ExternalInput")
    idxs = nc.dram_tensor("idxs", [NCHUNK, 128, IDX_COLS], mybir.dt.int16,
                          kind="ExternalInput")
    wm = nc.dram_tensor("wm", [TILES_PER_CORE * NCHUNK, 128, TILE], f32,
                        kind="ExternalInput")
    y = nc.dram_tensor("y", [ROWS_PER_CORE, D], f32, kind="ExternalOutput")

    with TileContext(nc) as tc:
        with (
            tc.tile_pool(name="const", bufs=1) as cpool,
            tc.tile_pool(name="g", bufs=2) as gpool,
            tc.tile_pool(name="w", bufs=2) as wpool,
            tc.tile_pool(name="o", bufs=2) as opool,
            tc.tile_pool(name="ps", bufs=2, space="PSUM") as ppool,
        ):
            idx_sb = cpool.tile([128, NCHUNK * IDX_COLS], mybir.dt.int16)
            for c in range(NCHUNK):
                nc.sync.dma_start(
                    out=idx_sb[:, c * IDX_COLS:(c + 1) * IDX_COLS],
                    in_=idxs[c, :, :])

            WTC = GROUP_TILES * NCHUNK          # 20 W tiles per group
            for g in range(GROUPS):
                G = []
                for c in range(NCHUNK):
                    gt = gpool.tile([128, GROUP_TILES * D], f32, tag=f"G{c}")
                    nc.gpsimd.dma_gather(
                        gt[:].rearrange("p (s d) -> p s d", d=D),
                        x[c * CHUNK:(c + 1) * CHUNK, :],
                        idx_sb[:, c * IDX_COLS + g * (GIDX // 16):
                               c * IDX_COLS + (g + 1) * (GIDX // 16)],
                        GIDX, GIDX, D, queue_num=c)
                    G.append(gt)
                wsb = wpool.tile([128, WTC * TILE], f32, tag="Ws")
                nc.sync.dma_start(
                    out=wsb[:].rearrange("p (t j) -> p t j", j=TILE),
                    in_=wm[g * WTC:(g + 1) * WTC].rearrange("t p j -> p t j"))
                if MM_DTYPE == "float32r":
                    wr = wpool.tile([128, WTC * TILE], mmdt, tag="Wr")
                    nc.vector.tensor_copy(wr[:], wsb[:])
                    Gm = []
                    for c in range(NCHUNK):
                        gr = gpool.tile([128, GROUP_TILES * D], mmdt,
                                        tag=f"Gr{c}")
                        nc.vector.tensor_copy(gr[:], G[c][:])
                        Gm.append(gr)
                else:
                    wr = wsb
                    Gm = G
                ostage = opool.tile([128, GROUP_TILES * D], f32, tag="out")
                for t5 in range(GROUP_TILES):
                    ps = ppool.tile([128, D], f32, tag="ps")
                    for c in range(NCHUNK):
                        wcol = (t5 * NCHUNK + c) * TILE
                        nc.tensor.matmul(
                            ps[:],
                            lhsT=wr[:, wcol:wcol + TILE],
                            rhs=Gm[c][:, t5 * D:(t5 + 1) * D],
                            start=(c == 0), stop=(c == NCHUNK - 1))
                    nc.scalar.copy(ostage[:, t5 * D:(t5 + 1) * D], ps[:])
                nc.sync.dma_start(
                    out=y[g * GIDX:(g + 1) * GIDX, :].rearrange(
                        "(t p) d -> p t d", p=128),
                    in_=ostage[:].rearrange("p (t d) -> p t d", d=D))
    nc.compile()
    return nc


def _get_compiled():
    global _COMPILED
    if _COMPILED is None:
        _COMPILED = _build()
    return _COMPILED


# -------------------------------------------------------------------- entry
def kernel(x, vals, rows, cols):
    x = np.ascontiguousarray(np.asarray(x, dtype=np.float32))
    shards = _plan(rows, cols, vals)
    nc = _get_compiled()

    from concourse.bass_utils import run_bass_kernel_spmd
    in_maps = [
        {"x": x, "idxs": s["idxs"], "wm": s["wm"]}
        for s in shards
    ]
    res = run_bass_kernel_spmd(nc, in_maps, core_ids=list(range(NCORES)))

    out = np.zeros((M_COARSE, D), np.float32)
    for k, s in enumerate(shards):
        yk = res.results[k]["y"]
        valid = s["m_of"] >= 0
        out[s["m_of"][valid]] = yk[valid]
    return out
